# revision 1
# baseline (speedup 1.0000x reference)
"""KMeans inference (argmin over squared distances) on 8 Trainium2 cores.

Problem: features [262144, 768] fp32, cluster_centers [1024, 768] fp32.
Output: argmin_k ||x_i - c_k||^2 as int32 [262144].

Strategy (data-parallel over rows):
  - argmin_k ||x-c_k||^2 == argmax_k (x.c_k - 0.5*||c_k||^2); the ||x||^2
    term is constant per row and drops out of the argmin.
  - Shard rows across 8 cores (32768 rows/core). Host pre-transposes each
    shard to xT [768, 32768] so the contraction dim (d) lands on SBUF
    partitions with fully contiguous DMA lines.
  - Per core: scores[m, k] = sum_d xT[d, m] * cT[d, k] via PE matmuls in
    fp32r (full-rate fp32-storage matmul). Both 512-wide k-halves stream
    under one stationary load so LDWEIGHTS stays hidden.
  - Scores are copied PSUM->SBUF with a cast to fp16 (centered so the
    fp16 ulp stays ~0.06), bias-added on DVE in fp16 (2x element rate),
    then argmax'd with the DVE MAX8/FIND_INDEX8 instructions.
  - Device also exports each row's top-2 score values. Rows whose top-2
    gap is under a threshold bounding the fp32r+fp16 error get an exact
    fp32 recompute on the host (~2% of rows), making the argmin exact.
"""

import sys

sys.path.insert(0, "/opt/trn_rl_repo")

import numpy as np

N_CORES = 8
N, K, D = 262144, 1024, 768
ROWS_PER_CORE = N // N_CORES          # 32768
SLAB_ROWS = 512                        # rows fetched per DMA slab
N_SLABS = ROWS_PER_CORE // SLAB_ROWS   # 64
SUBTILES = SLAB_ROWS // 128            # 4 row-tiles of 128 per slab
N_ROWTILES = ROWS_PER_CORE // 128      # 256
D_TILES = D // 128                     # 6
OUT_CHUNK_SLABS = 8                    # stream staging out every 8 slabs

# Score error budget: fp32r matmul |err| < ~3e-2, fp16 rounding of the
# centered score (|s| mostly < 70, ulp 0.0625) < ~3.1e-2, fp16 bias +
# add rounding < ~5e-2  =>  per-score |err| < ~0.12, top-2 gap error
# < ~0.24.  GAP_THRESHOLD = 0.35 covers it with margin.
GAP_THRESHOLD = 0.35
CENTER = 384.0  # ~E[0.5*||c_k||^2] for unit-variance d=768 centroids

_PROGRAM = None


def _build_program():
    import concourse.mybir as mybir
    from concourse import bacc
    from concourse.tile import TileContext

    F32 = mybir.dt.float32
    F32R = mybir.dt.float32r
    F16 = mybir.dt.float16
    U32 = mybir.dt.uint32

    nc = bacc.Bacc()
    # Inputs (per core): transposed feature shard, transposed centroids,
    # fp16 bias tile (CENTER - 0.5*||c_k||^2, replicated over partitions).
    xt = nc.declare_dram_parameter("xt", [D, ROWS_PER_CORE], F32R, isOutput=False)
    cbt = nc.declare_dram_parameter("cbt", [D, K], F32R, isOutput=False)
    bias = nc.declare_dram_parameter("bias", [128, K], F16, isOutput=False)
    # Outputs: idx[p, m] = argmax index of row m*128 + p; top2[p, 2m:2m+2]
    # = top-2 (fp16, centered) score values of that row.
    out_idx = nc.declare_dram_parameter("idx", [128, N_ROWTILES], U32, isOutput=True)
    out_top2 = nc.declare_dram_parameter(
        "top2", [128, 2 * N_ROWTILES], F16, isOutput=True
    )

    with TileContext(nc) as tc:
        with (
            tc.tile_pool(name="consts", bufs=1) as consts,
            tc.tile_pool(name="xslab", bufs=3) as xslab_pool,
            tc.tile_pool(name="scores", bufs=4) as scores_pool,
            tc.tile_pool(name="maxes", bufs=8) as maxes_pool,
            tc.tile_pool(name="stage", bufs=2) as stage_pool,
            tc.tile_pool(name="psum", bufs=4, space="PSUM") as psum_pool,
        ):
            # Centroids resident in SBUF: 6 tiles [128, 1024] + bias tile.
            cb = consts.tile([128, D_TILES, K], F32R, tag="cb")
            nc.sync.dma_start(
                out=cb,
                in_=cbt.rearrange("(t p) k -> p t k", p=128),
            )
            bias_t = consts.tile([128, K], F16, tag="bias")
            nc.sync.dma_start(out=bias_t, in_=bias[:, :])

            chunk_rt = OUT_CHUNK_SLABS * SUBTILES  # 32 row-tiles per chunk
            staging_idx = None

            for slab in range(N_SLABS):
                r0 = slab * SLAB_ROWS
                if slab % OUT_CHUNK_SLABS == 0:
                    staging_idx = stage_pool.tile([128, chunk_rt], U32, tag="sidx")
                    staging_top2 = stage_pool.tile(
                        [128, 2 * chunk_rt], F16, tag="stop2"
                    )
                xs = xslab_pool.tile([128, D_TILES, SLAB_ROWS], F32R, tag="xs")
                nc.sync.dma_start(
                    out=xs,
                    in_=xt.rearrange("(t p) r -> p t r", p=128)[
                        :, :, r0 : r0 + SLAB_ROWS
                    ],
                )
                for sub in range(SUBTILES):
                    mc = (slab % OUT_CHUNK_SLABS) * SUBTILES + sub
                    ps0 = psum_pool.tile([128, 512], F32, tag="ps0")
                    ps1 = psum_pool.tile([128, 512], F32, tag="ps1")
                    for dt in range(D_TILES):
                        xst = xs[:, dt, sub * 128 : (sub + 1) * 128]
                        nc.tensor.matmul(
                            ps0,
                            xst,
                            cb[:, dt, 0:512],
                            start=(dt == 0),
                            stop=(dt == D_TILES - 1),
                        )
                        nc.tensor.matmul(
                            ps1,
                            xst,
                            cb[:, dt, 512:1024],
                            start=(dt == 0),
                            stop=(dt == D_TILES - 1),
                        )
                    scores = scores_pool.tile([128, K], F16, tag="scores")
                    nc.scalar.copy(scores[:, 0:512], ps0)
                    nc.scalar.copy(scores[:, 512:1024], ps1)
                    # fp16 bias add (includes +CENTER) at 2x DVE rate
                    nc.vector.tensor_add(scores, scores, bias_t)
                    max8 = maxes_pool.tile([128, 8], F16, tag="max8")
                    idx8 = maxes_pool.tile([128, 8], U32, tag="idx8")
                    nc.vector.max(out=max8, in_=scores)
                    nc.vector.max_index(out=idx8, in_max=max8, in_values=scores)
                    nc.scalar.copy(staging_idx[:, mc : mc + 1], idx8[:, 0:1])
                    nc.scalar.copy(
                        staging_top2[:, 2 * mc : 2 * mc + 2], max8[:, 0:2]
                    )
                if slab % OUT_CHUNK_SLABS == OUT_CHUNK_SLABS - 1:
                    m0 = (slab - OUT_CHUNK_SLABS + 1) * SUBTILES
                    nc.sync.dma_start(
                        out=out_idx[:, m0 : m0 + chunk_rt], in_=staging_idx
                    )
                    nc.sync.dma_start(
                        out=out_top2[:, 2 * m0 : 2 * m0 + 2 * chunk_rt],
                        in_=staging_top2,
                    )

    nc.finalize()
    return nc


def _get_program():
    global _PROGRAM
    if _PROGRAM is None:
        _PROGRAM = _build_program()
    return _PROGRAM


def _make_in_maps(features, cluster_centers):
    cbt = np.ascontiguousarray(cluster_centers.T)  # [768, 1024]
    c2 = (cluster_centers.astype(np.float64) ** 2).sum(axis=1)
    bias_row = (CENTER - 0.5 * c2).astype(np.float16)
    bias = np.ascontiguousarray(np.broadcast_to(bias_row, (128, K)))

    in_maps = []
    for i in range(N_CORES):
        shard = features[i * ROWS_PER_CORE : (i + 1) * ROWS_PER_CORE]
        xtr = np.ascontiguousarray(shard.T)  # [768, 32768]
        in_maps.append({"xt": xtr, "cbt": cbt, "bias": bias})
    return in_maps


def _postprocess(res, features, cluster_centers):
    """Assemble indices; exactly recompute rows with a small top-2 gap."""
    idx_parts = []
    gap_parts = []
    for i in range(N_CORES):
        idx = res.results[i]["idx"]          # [128, 256] uint32
        top2 = res.results[i]["top2"]        # [128, 512] fp16
        idx_parts.append(idx.T.reshape(-1))  # row r = m*128 + p
        t2 = (
            top2.astype(np.float32)
            .reshape(128, N_ROWTILES, 2)
            .transpose(1, 0, 2)
            .reshape(-1, 2)
        )
        gap_parts.append(t2[:, 0] - t2[:, 1])
    out = np.concatenate(idx_parts).astype(np.int32)
    gap = np.concatenate(gap_parts)

    risky = np.flatnonzero(gap < GAP_THRESHOLD)
    if risky.size:
        x = features[risky]
        s = x @ cluster_centers.T
        s += -0.5 * (cluster_centers * cluster_centers).sum(axis=1)
        out[risky] = s.argmax(axis=1).astype(np.int32)
    return out


def kernel(features: np.ndarray, cluster_centers: np.ndarray) -> np.ndarray:
    from concourse.bass_utils import run_bass_kernel_spmd

    features = np.ascontiguousarray(features, dtype=np.float32)
    cluster_centers = np.ascontiguousarray(cluster_centers, dtype=np.float32)

    in_maps = _make_in_maps(features, cluster_centers)
    nc = _get_program()
    res = run_bass_kernel_spmd(nc, in_maps, core_ids=list(range(N_CORES)))
    return _postprocess(res, features, cluster_centers)


if __name__ == "__main__":
    rng = np.random.default_rng(0)
    f = rng.standard_normal((N, D)).astype(np.float32)
    c = rng.standard_normal((K, D)).astype(np.float32)
    got = kernel(f, c)
    d2 = (
        (f**2).sum(1, keepdims=True)
        - 2.0 * f @ c.T
        + (c**2).sum(1)
    )
    want = d2.argmin(1)
    print("mismatches:", (got != want).sum(), "/", N)



# revision 3
# speedup vs baseline: 1.0833x; 1.0833x over previous
"""KMeans inference (argmin over squared distances) on 8 Trainium2 cores.

Problem: features [262144, 768] fp32, cluster_centers [1024, 768] fp32.
Output: argmin_k ||x_i - c_k||^2 as int32 [262144].

Strategy (data-parallel over rows):
  - argmin_k ||x-c_k||^2 == argmax_k (x.c_k - 0.5*||c_k||^2); the ||x||^2
    term is constant per row and drops out of the argmin.
  - Shard rows across 8 cores (32768 rows/core). Host pre-casts to fp16 and
    pre-transposes each shard to xT [768, 32768] so the contraction dim (d)
    lands on SBUF partitions with contiguous DMA lines. fp16 halves the HBM
    traffic and enables fast weight load (FWL) on the PE.
  - Per core: scores[m, k] = sum_d xT[d, m] * cT[d, k] via fp16 PE matmuls
    (full rate, fp32 PSUM accumulation).
  - ACT copies PSUM->SBUF (fp32 scores); GPSIMD (otherwise idle) adds the
    -0.5*||c_k||^2 bias; DVE does MAX8 + FIND_INDEX8 writing top-8
    values/indices directly into the staging tiles that DMA out.
  - Host exactly (fp64) recomputes rows whose top-2 gap is below a threshold
    bounding the fp16 quantization error (~1% of rows), making the argmin
    exact for the fp32 problem.
"""

import sys

sys.path.insert(0, "/opt/trn_rl_repo")

import numpy as np

N_CORES = 8
N, K, D = 262144, 1024, 768
ROWS_PER_CORE = N // N_CORES          # 32768
SLAB_ROWS = 1024                       # rows fetched per DMA slab
N_SLABS = ROWS_PER_CORE // SLAB_ROWS   # 32
SUBTILES = SLAB_ROWS // 128            # 8 row-tiles of 128 per slab
N_ROWTILES = ROWS_PER_CORE // 128      # 256
D_TILES = D // 128                     # 6
OUT_CHUNK_SLABS = 4                    # stage outputs every 4 slabs

# Score error budget: fp16 quantization of x and c gives per-score error
# std ~0.005 (the c-side quantization is folded exactly into the bias, so
# only cross terms remain); fp32 accumulation noise ~1e-4. Top-2 gap error
# < ~0.02 at 3 sigma. GAP_THRESHOLD = 0.10 covers it with wide margin.
GAP_THRESHOLD = 0.10

_PROGRAM = None


def _build_program():
    import concourse.mybir as mybir
    from concourse import bacc
    from concourse.tile import TileContext

    F32 = mybir.dt.float32
    F16 = mybir.dt.float16
    U32 = mybir.dt.uint32

    nc = bacc.Bacc()
    # Inputs (per core): transposed fp16 feature shard, transposed fp16
    # centroids, fp32 bias tile (-0.5*||c_k||^2, replicated over partitions).
    xt = nc.declare_dram_parameter("xt", [D, ROWS_PER_CORE], F16, isOutput=False)
    cbt = nc.declare_dram_parameter("cbt", [D, K], F16, isOutput=False)
    bias = nc.declare_dram_parameter("bias", [128, K], F32, isOutput=False)
    # Outputs: idx8[p, 8m:8m+8] / val8[p, 8m:8m+8] = top-8 indices / biased
    # score values of row m*128 + p, descending.
    out_idx = nc.declare_dram_parameter(
        "idx8", [128, 8 * N_ROWTILES], U32, isOutput=True
    )
    out_val = nc.declare_dram_parameter(
        "val8", [128, 8 * N_ROWTILES], F32, isOutput=True
    )

    with TileContext(nc) as tc:
        with (
            tc.tile_pool(name="consts", bufs=1) as consts,
            tc.tile_pool(name="xslab", bufs=3) as xslab_pool,
            tc.tile_pool(name="scores", bufs=4) as scores_pool,
            tc.tile_pool(name="stage", bufs=2) as stage_pool,
            tc.tile_pool(name="psum", bufs=4, space="PSUM") as psum_pool,
        ):
            # Centroids resident in SBUF: 6 tiles [128, 1024] fp16 + bias.
            cb = consts.tile([128, D_TILES, K], F16, tag="cb")
            nc.sync.dma_start(
                out=cb,
                in_=cbt.rearrange("(t p) k -> p t k", p=128),
            )
            bias_t = consts.tile([128, K], F32, tag="bias")
            nc.sync.dma_start(out=bias_t, in_=bias[:, :])

            chunk_rt = OUT_CHUNK_SLABS * SUBTILES  # 32 row-tiles per chunk
            stage_idx = stage_val = None

            for slab in range(N_SLABS):
                r0 = slab * SLAB_ROWS
                if slab % OUT_CHUNK_SLABS == 0:
                    stage_idx = stage_pool.tile([128, 8 * chunk_rt], U32, tag="sidx")
                    stage_val = stage_pool.tile([128, 8 * chunk_rt], F32, tag="sval")
                xs = xslab_pool.tile([128, D_TILES, SLAB_ROWS], F16, tag="xs")
                nc.sync.dma_start(
                    out=xs,
                    in_=xt.rearrange("(t p) r -> p t r", p=128)[
                        :, :, r0 : r0 + SLAB_ROWS
                    ],
                )
                for sub in range(SUBTILES):
                    mc = (slab % OUT_CHUNK_SLABS) * SUBTILES + sub
                    ps0 = psum_pool.tile([128, 512], F32, tag="ps0")
                    ps1 = psum_pool.tile([128, 512], F32, tag="ps1")
                    for dt in range(D_TILES):
                        xst = xs[:, dt, sub * 128 : (sub + 1) * 128]
                        nc.tensor.matmul(
                            ps0,
                            xst,
                            cb[:, dt, 0:512],
                            start=(dt == 0),
                            stop=(dt == D_TILES - 1),
                        )
                        nc.tensor.matmul(
                            ps1,
                            xst,
                            cb[:, dt, 512:1024],
                            start=(dt == 0),
                            stop=(dt == D_TILES - 1),
                        )
                    scores = scores_pool.tile([128, K], F32, tag="scores")
                    nc.scalar.copy(scores[:, 0:512], ps0)
                    nc.scalar.copy(scores[:, 512:1024], ps1)
                    # bias add on the otherwise-idle GPSIMD engine
                    nc.gpsimd.tensor_add(scores, scores, bias_t)
                    v8 = stage_val[:, 8 * mc : 8 * mc + 8]
                    i8 = stage_idx[:, 8 * mc : 8 * mc + 8]
                    nc.vector.max(out=v8, in_=scores)
                    nc.vector.max_index(out=i8, in_max=v8, in_values=scores)
                if slab % OUT_CHUNK_SLABS == OUT_CHUNK_SLABS - 1:
                    m0 = (slab - OUT_CHUNK_SLABS + 1) * SUBTILES
                    nc.sync.dma_start(
                        out=out_idx[:, 8 * m0 : 8 * (m0 + chunk_rt)], in_=stage_idx
                    )
                    nc.sync.dma_start(
                        out=out_val[:, 8 * m0 : 8 * (m0 + chunk_rt)], in_=stage_val
                    )

    nc.finalize()
    return nc


def _get_program():
    global _PROGRAM
    if _PROGRAM is None:
        _PROGRAM = _build_program()
    return _PROGRAM


def _make_in_maps(features, cluster_centers):
    c16 = cluster_centers.astype(np.float16)
    cbt = np.ascontiguousarray(c16.T)  # [768, 1024] fp16
    # Exact norms of the fp16 centroids the device actually uses: the c-side
    # quantization is then part of the problem definition, not an error term.
    c2 = (c16.astype(np.float64) ** 2).sum(axis=1)
    bias_row = (-0.5 * c2).astype(np.float32)
    bias = np.ascontiguousarray(np.broadcast_to(bias_row, (128, K)))

    x16t = features.astype(np.float16).T  # [768, 262144] view
    in_maps = []
    for i in range(N_CORES):
        xtr = np.ascontiguousarray(
            x16t[:, i * ROWS_PER_CORE : (i + 1) * ROWS_PER_CORE]
        )
        in_maps.append({"xt": xtr, "cbt": cbt, "bias": bias})
    return in_maps


def _postprocess(res, features, cluster_centers):
    """Assemble indices; exactly recompute rows with a small top-2 gap."""
    idx_parts = []
    gap_parts = []
    for i in range(N_CORES):
        idx8 = res.results[i]["idx8"].reshape(128, N_ROWTILES, 8)
        val8 = res.results[i]["val8"].reshape(128, N_ROWTILES, 8)
        # row r = m*128 + p
        idx_parts.append(idx8[:, :, 0].T.reshape(-1))
        gap_parts.append(
            (val8[:, :, 0] - val8[:, :, 1]).astype(np.float32).T.reshape(-1)
        )
    out = np.concatenate(idx_parts).astype(np.int32)
    gap = np.concatenate(gap_parts)

    risky = np.flatnonzero(gap < GAP_THRESHOLD)
    if risky.size:
        x = features[risky].astype(np.float64)
        c = cluster_centers.astype(np.float64)
        s = x @ c.T
        s -= 0.5 * (c * c).sum(axis=1)
        out[risky] = s.argmax(axis=1).astype(np.int32)
    return out


def kernel(features: np.ndarray, cluster_centers: np.ndarray) -> np.ndarray:
    from concourse.bass_utils import run_bass_kernel_spmd

    features = np.ascontiguousarray(features, dtype=np.float32)
    cluster_centers = np.ascontiguousarray(cluster_centers, dtype=np.float32)

    in_maps = _make_in_maps(features, cluster_centers)
    nc = _get_program()
    res = run_bass_kernel_spmd(nc, in_maps, core_ids=list(range(N_CORES)))
    return _postprocess(res, features, cluster_centers)


if __name__ == "__main__":
    rng = np.random.default_rng(0)
    f = rng.standard_normal((N, D)).astype(np.float32)
    c = rng.standard_normal((K, D)).astype(np.float32)
    got = kernel(f, c)
    d2 = (
        (f**2).sum(1, keepdims=True)
        - 2.0 * f @ c.T
        + (c**2).sum(1)
    )
    want = d2.argmin(1)
    print("mismatches:", (got != want).sum(), "/", N)
